# revision 10
# baseline (speedup 1.0000x reference)
"""CRF loss kernel for Trainium2 (8 NeuronCores, batch-parallel).

Math: reference computes
  proj = em @ W + b                       (B,S,N)
  gold-path score + log-partition via forward scan over S
  loss = -sum(score - partition);  logits = proj[:, :, :33]

Device strategy (per core, 8 sequences):
  1. proj^T = W^T @ em^T via PE (bf16 operands, fp32 PSUM).  em^T is
     pre-transposed and cast on host (layout/sharding prep).
  2. EXPE = exp(proj^T + b) via ACT (bias is per-partition in this layout).
  3. Forward scan in exp space: alpha_{t+1} = (alpha_t @ expT) * exp(em'_t).
     exp(T)/N is constant => chunked scan: 16 chunks x 64 steps advance in
     lockstep; each chunk evolves a 34x34 transfer matrix (entering-state i
     on columns).  State VT[j, (pair,i)] packed 3 chunk-groups deep on the
     partition axis (102 rows) with a block-diagonal expT weight matrix so
     matmul + DVE both use ~102 lanes.  Per step: one fp32r matmul set +
     one DVE multiply by the per-(pair,step) exp(proj') column scales.
  4. Outputs: proj^T (f32) and final chunk-matrix state.  Host combines the
     16 chunk matrices per sequence (fp64), computes the gold score from
     logits (exact), and sums the loss.
"""

import numpy as np
import ml_dtypes

import concourse.bass as bass
import concourse.bacc as bacc
import concourse.mybir as mybir
from concourse.tile import TileContext
from concourse.bass_utils import run_bass_kernel_spmd

B, S, D = 64, 1024, 1024
NUM_TAGS = 33
N = 34
BOS = N - 1
NEG_INF = -10000.0
NCORES = 8
BL = B // NCORES          # 8 sequences per core
K = 16                    # chunks per sequence
L = S // K                # 64 steps per chunk (step 0 is the init step)
NPAIR = BL * K            # 128 (seq, chunk) pairs per core
NTRIP = (NPAIR + 2) // 3  # 43 column triples
G = 3                     # partition bands
PB = G * N                # 102 partitions used
BS = BL * S               # 8192 rows per core

_CACHE = {}


def _build_bass():
    nc = bacc.Bacc()
    emT = nc.declare_dram_parameter("emT", [D, BS], mybir.dt.bfloat16, isOutput=False).ap()
    Wb = nc.declare_dram_parameter("Wb", [D, N], mybir.dt.bfloat16, isOutput=False).ap()
    eT3h = nc.declare_dram_parameter("eT3h", [PB, PB], mybir.dt.bfloat16, isOutput=False).ap()
    eT3l = nc.declare_dram_parameter("eT3l", [PB, PB], mybir.dt.bfloat16, isOutput=False).ap()
    vt0 = nc.declare_dram_parameter("vt0", [PB, NTRIP * N], mybir.dt.bfloat16, isOutput=False).ap()
    zpad = nc.declare_dram_parameter("zpad", [N, L], mybir.dt.float32, isOutput=False).ap()
    projT = nc.declare_dram_parameter("projT", [N, BS], mybir.dt.float32, isOutput=True).ap()
    vtout = nc.declare_dram_parameter("vtout", [PB, NTRIP * N], mybir.dt.bfloat16, isOutput=True).ap()

    f32 = mybir.dt.float32
    f32r = mybir.dt.float32r
    bf16 = mybir.dt.bfloat16
    EXP = mybir.ActivationFunctionType.Exp

    with TileContext(nc) as tc:
        with (
            tc.tile_pool(name="const", bufs=1) as cpool,
            tc.tile_pool(name="work", bufs=3) as wpool,
            tc.tile_pool(name="state", bufs=1) as spool,
            tc.tile_pool(name="psum", bufs=2, space="PSUM") as ppool,
            tc.tile_pool(name="psumL", bufs=4, space="PSUM") as ppoolL,
        ):
            # ---- constants ----
            w_sb = cpool.tile([128, 8, N], bf16, tag="w")
            nc.sync.dma_start(w_sb[:], Wb.rearrange("(c p) n -> p c n", p=128))
            eT3h_sb = cpool.tile([PB, PB], bf16, tag="eT3h")
            nc.sync.dma_start(eT3h_sb[:], eT3h)
            eT3l_sb = cpool.tile([PB, PB], bf16, tag="eT3l")
            nc.sync.dma_start(eT3l_sb[:], eT3l)

            # ---- emissions (transposed bf16) ----
            em_tiles = []
            for c in range(8):
                t = cpool.tile([128, BS], bf16, tag=f"emT{c}")
                nc.sync.dma_start(t[:], emT[c * 128:(c + 1) * 128, :])
                em_tiles.append(t)

            # ---- projection + exp ----
            expe = spool.tile([N, BS], f32, tag="expe")
            for i in range(BS // 128):
                ps = ppool.tile([N, 128], f32, tag="proj")
                for c in range(8):
                    nc.tensor.matmul(
                        ps[:],
                        lhsT=w_sb[:, c, :],
                        rhs=em_tiles[c][:, i * 128:(i + 1) * 128],
                        start=(c == 0),
                        stop=(c == 7),
                    )
                pj = wpool.tile([N, 128], f32, tag="pj")
                nc.vector.tensor_copy(pj[:], ps[:])
                nc.scalar.activation(
                    expe[:, i * 128:(i + 1) * 128], pj[:], EXP, bias=0.0
                )
                nc.gpsimd.dma_start(projT[:, i * 128:(i + 1) * 128], pj[:])

            # ---- band-replicate EXPE: band g holds pairs g, g+3, ... ----
            # expe free dim is (pair, tau) = (128, 64) row-major already.
            exb = spool.tile([PB, NTRIP, L], f32, tag="exb")
            expe_v = expe.rearrange("p (pr t) -> p pr t", pr=NPAIR)
            nc.gpsimd.dma_start(exb[2 * N:, NTRIP - 1, :], zpad)
            for g in range(G):
                cnt = NTRIP if g < 2 else NTRIP - 1
                nc.gpsimd.dma_start(
                    exb[g * N:(g + 1) * N, :cnt, :],
                    expe_v[:, g:NPAIR:3, :],
                )

            # ---- scan state ----
            vt_a = spool.tile([PB, NTRIP, N], bf16, tag="vt_a")
            vt_b = spool.tile([PB, NTRIP, N], bf16, tag="vt_b")
            nc.sync.dma_start(vt_a[:], vt0.rearrange("p (r n) -> p r n", n=N))

            groups = [(0, 15), (15, 30), (30, NTRIP)]
            cur, nxt = vt_a, vt_b
            for tau in range(L):
                for gi, (r0, r1) in enumerate(groups):
                    w = r1 - r0
                    ps = ppoolL.tile([PB, w, N], f32, tag="scan")
                    nc.tensor.matmul(
                        ps[:], lhsT=eT3h_sb[:], rhs=cur[:, r0:r1, :],
                        start=True, stop=False,
                    )
                    nc.tensor.matmul(
                        ps[:], lhsT=eT3l_sb[:], rhs=cur[:, r0:r1, :],
                        start=False, stop=True,
                    )
                    scale = exb[:, r0:r1, tau].unsqueeze(2).broadcast_to((PB, w, N))
                    nc.vector.tensor_mul(nxt[:, r0:r1, :], ps[:], scale)
                cur, nxt = nxt, cur

            nc.gpsimd.dma_start(vtout.rearrange("p (r n) -> p r n", n=N), cur[:])

    nc.compile()
    return nc


def _get_nc():
    if "nc" not in _CACHE:
        _CACHE["nc"] = _build_bass()
    return _CACHE["nc"]


def _host_inputs(emissions, W, b, transitions):
    bf = ml_dtypes.bfloat16
    expTs = (np.exp(transitions.astype(np.float64)) / N) * np.exp(
        b.astype(np.float64)
    )[None, :]
    eT3 = np.zeros((PB, PB), np.float64)
    for g in range(G):
        eT3[g * N:(g + 1) * N, g * N:(g + 1) * N] = expTs
    eT3h = eT3.astype(bf)
    eT3l = (eT3 - eT3h.astype(np.float64)).astype(bf)
    vt0 = np.zeros((PB, NTRIP * N), bf)
    eye = np.eye(N, dtype=bf)
    for g in range(G):
        for r in range(NTRIP):
            vt0[g * N:(g + 1) * N, r * N:(r + 1) * N] = eye
    Wb = np.ascontiguousarray(W.astype(bf))
    in_maps = []
    for m in range(NCORES):
        emc = emissions[m * BL:(m + 1) * BL].reshape(BS, D)
        emTm = np.ascontiguousarray(emc.T).astype(bf)
        in_maps.append(
            {"emT": emTm, "Wb": Wb, "eT3h": eT3h, "eT3l": eT3l,
             "vt0": vt0, "zpad": np.zeros((N, L), np.float32)}
        )
    return in_maps


def _combine_host(vt_all, proj_full, tags, mask, transitions, b):
    """vt_all: (NCORES, PB, NTRIP*N). proj_full: (B,S,N) with bias included."""
    logZ = np.zeros(B)
    for m in range(NCORES):
        vt = vt_all[m].reshape(PB, NTRIP, N).astype(np.float64)
        for bl in range(BL):
            v = np.zeros(N)
            v[BOS] = 1.0
            off = 0.0
            for c in range(K):
                pair = bl * K + c
                g, r = pair % G, pair // G
                M = vt[g * N:(g + 1) * N, r, :].T  # [i, nxt]
                v = v @ M
                mx = v.max()
                v /= mx
                off += np.log(mx)
            logZ[m * BL + bl] = np.log(v.sum()) + off + S * np.log(float(N))

    tags = tags.astype(np.int64)
    T64 = transitions.astype(np.float64)
    p64 = proj_full.astype(np.float64)
    first = tags[:, 0]
    score = T64[BOS, first] + p64[np.arange(B), 0, first]
    e_t = np.take_along_axis(p64[:, 1:], tags[:, 1:, None], axis=2)[..., 0]
    t_t = T64[tags[:, :-1], tags[:, 1:]]
    score = score + np.sum((e_t + t_t) * mask[:, 1:].astype(np.float64), axis=1)
    return -np.sum(score - logZ)


def _numpy_reference(emissions, tags, mask, W, b, transitions):
    em = emissions.astype(np.float64) @ W.astype(np.float64) + b.astype(np.float64)
    tags = tags.astype(np.int64)
    mask64 = mask.astype(np.float64)
    T64 = transitions.astype(np.float64)
    first = tags[:, 0]
    score = T64[BOS, first] + em[np.arange(B), 0, first]
    e_t = np.take_along_axis(em[:, 1:], tags[:, 1:, None], axis=2)[..., 0]
    t_t = T64[tags[:, :-1], tags[:, 1:]]
    score = score + np.sum((e_t + t_t) * mask64[:, 1:], axis=1)
    alphas = T64[BOS][None, :] + em[:, 0]
    for t in range(1, S):
        sc = alphas[:, :, None] + T64[None] + em[:, t][:, None, :]
        mx = sc.max(axis=1)
        new = mx + np.log(np.exp(sc - mx[:, None, :]).sum(axis=1))
        m = mask64[:, t][:, None]
        alphas = m * new + (1.0 - m) * alphas
    mx = alphas.max(axis=1)
    part = mx + np.log(np.exp(alphas - mx[:, None]).sum(axis=1))
    loss = -np.sum(score - part)
    logits = (em[:, :, :NUM_TAGS]).astype(np.float32)
    return np.float32(loss), logits


def kernel(emissions, tags, mask, W, b, transitions, _profile=False):
    emissions = np.asarray(emissions, np.float32)
    tags = np.asarray(tags)
    mask = np.asarray(mask, np.float32)
    W = np.asarray(W, np.float32)
    b = np.asarray(b, np.float32)
    transitions = np.asarray(transitions, np.float32)

    if not np.all(mask == 1.0):
        return _numpy_reference(emissions, tags, mask, W, b, transitions)

    nc = _get_nc()
    in_maps = _host_inputs(emissions, W, b, transitions)
    res = run_bass_kernel_spmd(nc, in_maps, list(range(NCORES)), trace=_profile)
    results = res.results

    projs = []
    vt_all = []
    for m in range(NCORES):
        pT = np.asarray(results[m]["projT"], np.float32)  # (N, BS)
        projs.append(np.ascontiguousarray(pT.T).reshape(BL, S, N))
        vt_all.append(np.asarray(results[m]["vtout"]).astype(np.float32))
    proj_full = np.concatenate(projs, axis=0) + b[None, None, :]

    loss = _combine_host(np.stack(vt_all), proj_full, tags, mask, transitions, b)
    logits = np.ascontiguousarray(proj_full[:, :, :NUM_TAGS]).astype(np.float32)
    out = (np.float32(loss), logits)
    if _profile:
        return out, res
    return out
